# revision 29
# baseline (speedup 1.0000x reference)
"""Trainium2 Bass kernel for CRKT layer (decay-reweighted causal attention), v3.

Math per batch b (one NeuronCore per batch element, 8 cores):
  q = query @ Wq.T + bq ; k = key_in @ Wq.T + bq ; v = value @ Wv.T + bv
  s = q k^T  (per head, causal; 1/sqrt(dk) folded into exp scales)
  expS = exp(0.125 s); Z = rowsum; scan = cumsum(expS)
  te = exp((lam/Z) * (scan - Z) * (i-j))     [= exp(-lam*dist)]
  P2 = exp(0.125 * s * te); alpha = P2 / rowsum(P2)
  out = alpha @ v ; y = LN(out @ Wo.T + bo) * gamma + beta

v3 structure (vs v2):
  - ACT (the bottleneck engine) does ONLY the 3 exps per (h,t) job in the
    main loop; all PSUM->SBUF copies moved to Pool, proj biases to DVE.
  - cumsum scan moved DVE -> Pool; stt single-op (scalar_tensor_tensor)
    split Pool (t<6) / DVE (t>=6).
  - causal-mask add matmul in bf16 (1 cyc/row instead of 4 for f32r@128).
  - prologue restructured for early pipeline start: Wq/Q/K loaded first
    (Q/K in halves), only the g=0 projection runs up front; everything
    else (other projections, V/Wo path) issues as paced closures inside
    the attention loop, gated on DMA arrival estimates to avoid engine
    queue head-of-line blocking.
"""

import sys

for _p in ("/opt/trn_rl_repo",):
    if _p not in sys.path:
        sys.path.insert(0, _p)

import numpy as np

import concourse.bass as bass
import concourse.mybir as mybir
import concourse.tile as tile
from concourse import bacc, bass_utils
from concourse.masks import make_identity

F32 = mybir.dt.float32
F32R = mybir.dt.float32r
BF16 = mybir.dt.bfloat16
F16 = mybir.dt.float16
AL = mybir.AluOpType
AF = mybir.ActivationFunctionType

S, DIM, H, DK = 1024, 512, 8, 64
T = S // 128        # 8 i-tiles
NB = S // 128       # 8 j-blocks
NEGBIG = -1e30

_CACHE = {}


def _chunks(total, step):
    return [(a, min(a + step, total)) for a in range(0, total, step)]


def build():
    nc = bacc.Bacc("TRN2", target_bir_lowering=False, debug=False, num_devices=8)

    d_query = nc.dram_tensor("query", [S, DIM], F32, kind="ExternalInput")
    d_key = nc.dram_tensor("key_in", [S, DIM], F32, kind="ExternalInput")
    d_value = nc.dram_tensor("value", [S, DIM], F32, kind="ExternalInput")
    d_wq = nc.dram_tensor("Wq", [DIM, DIM], F32, kind="ExternalInput")
    d_wv = nc.dram_tensor("Wv", [DIM, DIM], F32, kind="ExternalInput")
    d_wo = nc.dram_tensor("Wo", [DIM, DIM], F32, kind="ExternalInput")
    d_bq = nc.dram_tensor("bq", [1, DIM], F32, kind="ExternalInput")
    d_bv = nc.dram_tensor("bv", [1, DIM], F32, kind="ExternalInput")
    d_bo = nc.dram_tensor("bo", [1, DIM], F32, kind="ExternalInput")
    d_dec = nc.dram_tensor("decay", [1, H], F32, kind="ExternalInput")
    d_gam = nc.dram_tensor("gamma", [1, DIM], F32, kind="ExternalInput")
    d_bet = nc.dram_tensor("beta", [1, DIM], F32, kind="ExternalInput")
    d_out = nc.dram_tensor("out", [S, DIM], F32, kind="ExternalOutput")

    with tile.TileContext(nc) as tc:
        _body(nc, tc, d_query, d_key, d_value, d_wq, d_wv, d_wo,
              d_bq, d_bv, d_bo, d_dec, d_gam, d_bet, d_out)

    nc.compile()
    return nc


def _body(nc, tc, d_query, d_key, d_value, d_wq, d_wv, d_wo,
          d_bq, d_bv, d_bo, d_dec, d_gam, d_bet, d_out):
    import contextlib
    ctx = contextlib.ExitStack()
    with ctx:
        const = ctx.enter_context(tc.tile_pool(name="const", bufs=1))
        persist = ctx.enter_context(tc.tile_pool(name="persist", bufs=1))

        # ---- priority DMA loads: big attention-critical tensors first ----
        wa_q = persist.tile([128, 4 * DIM], F32, tag="waq", name="wa_q")
        nc.sync.dma_start(
            wa_q[:].rearrange("p (r d) -> p r d", d=DIM),
            d_wq.ap().rearrange("(r p) d -> p r d", p=128))

        xa_q = persist.tile([128, T * DIM], F32, tag="xaq", name="xa_q")
        xa_k = persist.tile([128, T * DIM], F32, tag="xak", name="xa_k")

        def load_x_half(xa, dram, half):
            t0 = 4 * half
            nc.sync.dma_start(
                xa[:, t0 * DIM:(t0 + 4) * DIM].rearrange(
                    "p (t d) -> p t d", d=DIM),
                dram[128 * 4 * half:128 * 4 * (half + 1), :].rearrange(
                    "(t p) d -> p t d", p=128))

        load_x_half(xa_q, d_query, 0)
        load_x_half(xa_k, d_key, 0)

        # small consts needed early
        lam = const.tile([128, H], F32)     # |decay_h| broadcast down partitions
        nc.sync.dma_start(lam[:], d_dec.ap().to_broadcast((128, H)))
        nc.scalar.activation(lam[:], lam[:], AF.Abs)
        bq_sb = const.tile([128, 4], F32)   # col m = bq[128m:128(m+1)]
        nc.sync.dma_start(bq_sb[:], d_bq[0, :].rearrange("(m p) -> p m", p=128))

        # remaining big loads
        wa_v = persist.tile([128, 4 * DIM], F32, tag="wav", name="wa_v")
        nc.sync.dma_start(
            wa_v[:].rearrange("p (r d) -> p r d", d=DIM),
            d_wv.ap().rearrange("(r p) d -> p r d", p=128))
        load_x_half(xa_q, d_query, 1)
        load_x_half(xa_k, d_key, 1)
        # xa_v aliases xa_q: V is loaded into it by a closure issued after
        # the xq-half1 transposes (WAR ordering), saving 16KB/partition.
        # wa_o aliases wa_q the same way (loaded after the Wq transposes).
        xa_v = xa_q
        wa_o = wa_q

        # late consts
        bv_pk = const.tile([128, 4], F32)   # col r = bv[128r:128(r+1)]
        nc.sync.dma_start(bv_pk[:], d_bv[0, :].rearrange("(m p) -> p m", p=128))
        bv_pkr = const.tile([128, 4], BF16)
        nc.vector.tensor_copy(bv_pkr[:], bv_pk[:])
        bo_rowr = const.tile([1, DIM], F32R)
        nc.gpsimd.dma_start(bo_rowr[:], d_bo[0:1, :])
        gam_bc = const.tile([128, DIM], F32)
        nc.sync.dma_start(gam_bc[:], d_gam.ap().to_broadcast((128, DIM)))
        bet_bc = const.tile([128, DIM], F32)
        nc.sync.dma_start(bet_bc[:], d_bet.ap().to_broadcast((128, DIM)))

        # ---- non-DMA constants ----
        # Single distance tile (Pool, before the first scans):
        # dall[p, c] = 896 + p - c; view for i-tile t is dall[:, 128(7-t):...]
        # so that dall[p, j + 128(7-t)] = 128t + p - j = i - j.
        dall = const.tile([128, S], F16, tag="dall", name="dall")
        nc.gpsimd.iota(dall[:], pattern=[[-1, S]], base=128 * (T - 1),
                       channel_multiplier=1,
                       allow_small_or_imprecise_dtypes=True)

        def d_view(t):
            off = 128 * (T - 1 - t)
            return dall[:, off:off + 128 * (t + 1)]

        ident = const.tile([128, 128], F32)
        make_identity(nc, ident[:])
        identb = const.tile([128, 128], BF16)
        nc.vector.tensor_copy(identb[:], ident[:])
        cmaskb = const.tile([128, 128], BF16)
        nc.gpsimd.memset(cmaskb[:], 0.0)
        nc.gpsimd.affine_select(
            out=cmaskb[:], in_=cmaskb[:], compare_op=AL.is_ge, fill=NEGBIG,
            base=0, channel_multiplier=1, pattern=[[-1, 128]])
        eps_t = const.tile([128, 1], F32)
        nc.vector.memset(eps_t[:], 1e-5)
        ones1 = const.tile([1, 128], F32R)
        nc.vector.memset(ones1[:], 1.0)
        onecol = ones1[:, 0:1]

        # ---- persistent tensors ----
        qt = [persist.tile([128, S], F32R, tag=f"qt{g}", name=f"qt{g}") for g in range(4)]
        kt = [persist.tile([128, S], F32R, tag=f"kt{g}", name=f"kt{g}") for g in range(4)]
        xt_q = [persist.tile([128, S], F32R, tag=f"xtq{d}", name=f"xtq{d}") for d in range(4)]
        xt_k = [persist.tile([128, S], F32R, tag=f"xtk{d}", name=f"xtk{d}") for d in range(4)]
        # xt_v tiles are allocated later inside the _xtvT closures, reusing
        # the xtq tags (same buffers; WAR deps handled by the tag ring).
        xt_v = [None] * 4
        wqt = [persist.tile([128, DIM], F32R, tag=f"wqt{c}", name=f"wqt{c}") for c in range(4)]
        wvt = [persist.tile([128, DIM], F32R, tag=f"wvt{c}", name=f"wvt{c}") for c in range(4)]
        v_sb = [persist.tile([128, DIM], BF16, tag=f"v{t}", name=f"v{t}") for t in range(T)]
        ot_sb = [persist.tile([128, S], BF16, tag=f"ot{g}", name=f"ot{g}") for g in range(4)]
        wot = [persist.tile([128, DIM], BF16, tag=f"wot{g}", name=f"wot{g}") for g in range(4)]
        b2row = None  # allocated in _b2, reusing the qt0 tag

        # ---- PSUM pools (8 banks total: 2 + 2*2 + 1 = 7 used) ----
        pssA = ctx.enter_context(tc.tile_pool(name="pssA", bufs=1, space="PSUM"))
        pssB = ctx.enter_context(tc.tile_pool(name="pssB", bufs=2, space="PSUM"))
        psot = ctx.enter_context(tc.tile_pool(name="psot", bufs=1, space="PSUM"))
        sbA = ctx.enter_context(tc.tile_pool(name="sbA", bufs=4))
        sbS = ctx.enter_context(tc.tile_pool(name="sbS", bufs=4))
        sbP = ctx.enter_context(tc.tile_pool(name="sbP", bufs=2))
        sbStr = ctx.enter_context(tc.tile_pool(name="sbStr", bufs=1))

        # ---- prologue helpers ----
        def transp_w(wa, wt_dst, c):
            # one column-block c of W^T into wt_dst[c]; copy on DVE
            pt = pssB.tile([128, S], F32, tag="sb", name=f"ptw_{c}")
            for r in range(4):
                nc.tensor.transpose(
                    pt[:, 128 * r:128 * (r + 1)],
                    wa[:, DIM * r + 128 * c:DIM * r + 128 * (c + 1)],
                    ident[:])
            nc.vector.tensor_copy(wt_dst[c][:], pt[:, :DIM])

        def transp_x_half(xa, xt, dblk, half, on_act):
            # transpose 4 t-blocks (one half) of input dblk; copy DVE or ACT
            ptx = pssB.tile([128, S], F32, tag="sb", name=f"ptx_{dblk}_{half}")
            for tt in range(4):
                t = 4 * half + tt
                nc.tensor.transpose(
                    ptx[:, 128 * tt:128 * (tt + 1)],
                    xa[:, DIM * t + 128 * dblk:DIM * t + 128 * (dblk + 1)],
                    ident[:])
            dst = xt[dblk][:, 512 * half:512 * (half + 1)]
            if on_act:
                nc.scalar.activation(dst, ptx[:, :512], AF.Identity)
            else:
                nc.vector.tensor_copy(dst, ptx[:, :512])

        def proj_qk_half(xt, dst_tiles, g, half):
            # qt/kt[g][:, 512h:512h+512] = Wq[g-block] @ x^T[:, half] + bq
            pp = pssB.tile([128, S], F32, tag="sb", name=f"pp_{g}_{half}")
            for kk in range(4):
                nc.tensor.matmul(
                    pp[:, :DIM],
                    wqt[kk][:, 128 * g:128 * (g + 1)],
                    xt[kk][:, DIM * half:DIM * (half + 1)],
                    start=(kk == 0), stop=(kk == 3))
            dst = dst_tiles[g][:, DIM * half:DIM * (half + 1)]
            nc.vector.tensor_scalar_add(dst, in0=pp[:, :DIM],
                                        scalar1=bq_sb[:, g:g + 1])

        # ---- upfront PE work: enough for pair 0, jobs t<4 ----
        for c in range(4):
            transp_w(wa_q, wqt, c)
        for dblk in range(4):
            transp_x_half(xa_q, xt_q, dblk, 0, on_act=False)
        for dblk in range(4):
            transp_x_half(xa_k, xt_k, dblk, 0, on_act=True)
        proj_qk_half(xt_q, qt, 0, 0)
        proj_qk_half(xt_k, kt, 0, 0)

        # Wo load into wa_q's buffer — issued after the Wq transposes above
        # so the WAR dependency is tracked correctly.
        nc.sync.dma_start(
            wa_o[:].rearrange("p (r d) -> p r d", d=DIM),
            d_wo.ap().rearrange("(r p) d -> p r d", p=128))

        # ---- paced closures for the rest of the prologue ----
        def _wvT(c):
            def run():
                transp_w(wa_v, wvt, c)
            return run

        def _xT1(xa, xt, dblk, on_act):
            def run():
                transp_x_half(xa, xt, dblk, 1, on_act)
            return run

        def _proj(xt, dst, g, half):
            def run():
                proj_qk_half(xt, dst, g, half)
            return run

        def _xtvT(dblk):
            def run():
                xt_v[dblk] = persist.tile([128, S], F32R, tag=f"xtq{dblk}",
                                          name=f"xtv{dblk}")
                ptx = pssB.tile([128, S], F32, tag="sb", name=f"ptxv{dblk}")
                for t in range(T):
                    nc.tensor.transpose(
                        ptx[:, 128 * t:128 * (t + 1)],
                        xa_v[:, DIM * t + 128 * dblk:DIM * t + 128 * (dblk + 1)],
                        ident[:])
                nc.vector.tensor_copy(xt_v[dblk][:], ptx[:])
            return run

        def _vproj(t):
            def run():
                pp = pssB.tile([128, S], F32, tag="sb", name=f"ppv{t}")
                for kk in range(4):
                    nc.tensor.matmul(
                        pp[:, :DIM], xt_v[kk][:, 128 * t:128 * (t + 1)],
                        wvt[kk][:],
                        start=(kk == 0), stop=(kk == 3))
                nc.gpsimd.tensor_copy(v_sb[t][:], pp[:, :DIM])
            return run

        def _wo(c):
            def run():
                pt = pssB.tile([128, S], F32, tag="sb", name=f"ptwo{c}")
                for r in range(4):
                    nc.tensor.transpose(
                        pt[:, 128 * r:128 * (r + 1)],
                        wa_o[:, DIM * r + 128 * c:DIM * r + 128 * (c + 1)],
                        ident[:])
                nc.gpsimd.tensor_copy(wot[c][:], pt[:, :DIM])
            return run

        def _b2():
            # b2 = Wo bv + bo (bv folds through PV: alpha rows sum to 1)
            nonlocal b2row
            b2full = persist.tile([128, S], F32R, tag="qt0", name="b2row")
            b2row = b2full[0:1, :DIM]
            b2t = pssB.tile([128, S], F32, tag="sb", name="b2ps")
            b2ps = b2t[0:1, :DIM]
            for g in range(4):
                nc.tensor.matmul(b2ps, bv_pkr[:, g:g + 1],
                                 wot[g][:], start=(g == 0), stop=False)
            nc.tensor.matmul(b2ps, onecol, bo_rowr[:],
                             start=False, stop=True)
            nc.vector.tensor_copy(b2row, b2ps)

        # (min_global_iter, closure); paced 3/iter when eligible.
        # DMA eta (us): wq 2.9, xq0 5.8, xk0 8.7, wv 11.6, xq1 14.5,
        # xk1 17.4, xv 23.2, wo 26.1. ACT-time(iter n) ~= 13 + work(jobs<n).
        # ALL xt_q/xt_k readers (every projection) must be issued before the
        # xtvT closures reuse the xtq-tag buffers.
        def _vload():
            nc.sync.dma_start(
                xa_v[:].rearrange("p (t d) -> p t d", d=DIM),
                d_value.ap().rearrange("(t p) d -> p t d", p=128))

        tail = []
        for c in range(4):
            tail.append((1, _wvT(c)))
        for d in range(4):
            tail.append((3, _xT1(xa_q, xt_q, d, False)))
        for d in range(4):
            tail.append((6, _xT1(xa_k, xt_k, d, False)))
        tail.append((6, _vload))
        tail.append((8, _proj(xt_q, qt, 0, 1)))
        tail.append((8, _proj(xt_k, kt, 0, 1)))
        for g in range(1, 4):
            for half in range(2):
                tail.append((8 + g, _proj(xt_q, qt, g, half)))
                tail.append((8 + g, _proj(xt_k, kt, g, half)))
        for d in range(4):
            tail.append((13, _xtvT(d)))
        for t in range(T):
            tail.append((15, _vproj(t)))
        for c in range(4):
            tail.append((17, _wo(c)))
        tail.append((18, _b2))

        # ---- attention ----
        # strided views over packed strips: strips_cX[h][:, b, i'] (b-major)
        strips_c0 = {}
        strips_c1 = {}

        def qk(h, t, ps):
            g, off = h // 2, 64 * (h % 2)
            L = 128 * (t + 1)
            for (j0, j1) in _chunks(L, 512):
                nc.tensor.matmul(
                    ps[:, j0:j1],
                    qt[g][off:off + 64, 128 * t:128 * (t + 1)],
                    kt[g][off:off + 64, j0:j1],
                    start=True, stop=(j1 < L))
            nc.tensor.matmul(ps[:, 128 * t:L], identb[:], cmaskb[:],
                             start=False, stop=True)

        st = {}

        def stage_exp1(h, t, par):
            L = 128 * (t + 1)
            ps = pssA.tile([128, S], F32, tag="sa", name=f"psa_{h}_{t}")
            qk(h, t, ps)
            expS = sbA.tile([128, S], F32, tag=f"expS{par}",
                            name=f"e_{h}_{t}", bufs=2)
            nc.scalar.activation(expS[:, :L], ps[:, :L], AF.Exp,
                                 scale=0.125)
            st[(h, t)] = {"expS": expS}

        def stage_scan(h, t, par):
            L = 128 * (t + 1)
            d = st[(h, t)]
            scan = d["expS"]
            nc.gpsimd.tensor_tensor_scan(
                scan[:, :L], d["expS"][:, :L], d["expS"][:, :L], 0.0,
                op0=AL.add, op1=AL.bypass)
            sc1 = sbS.tile([128, 1], F32, tag="sc1", name=f"s1_{h}_{t}")
            z1 = scan[:, L - 1:L]
            nc.vector.reciprocal(sc1[:], z1)
            nc.vector.tensor_scalar_mul(sc1[:], in0=sc1[:],
                                        scalar1=lam[:, h:h + 1])
            d["sc1"] = sc1
            # stt: u = (scan - Z) * d (in place), single op
            eng = nc.gpsimd if t < 6 else nc.vector
            eng.scalar_tensor_tensor(
                scan[:, :L], in0=scan[:, :L], scalar=z1,
                in1=d_view(t), op0=AL.subtract, op1=AL.mult)

        def stage_te(h, t, par):
            L = 128 * (t + 1)
            d = st[(h, t)]
            te = sbP.tile([128, S], BF16, tag=f"te{par}",
                          name=f"te_{h}_{t}", bufs=1)
            nc.scalar.activation(te[:, :L], d["expS"][:, :L], AF.Exp,
                                 scale=d["sc1"][:])
            d["te"] = te

        def stage_s2(h, t, par):
            L = 128 * (t + 1)
            d = st[(h, t)]
            ps = pssB.tile([128, S], F32, tag="sb", name=f"psb_{h}_{t}")
            qk(h, t, ps)
            nc.vector.tensor_mul(ps[:, :L], ps[:, :L], d["te"][:, :L])
            d["ps"] = ps

        def stage_exp2(h, t, par):
            L = 128 * (t + 1)
            d = st[(h, t)]
            p2 = sbA.tile([128, S], BF16, tag=f"p2{par}",
                          name=f"p2_{h}_{t}", bufs=1)
            z2 = sbS.tile([128, 1], F32, tag="z2", name=f"z2_{h}_{t}")
            nc.scalar.activation(p2[:, :L], d["ps"][:, :L], AF.Exp,
                                 scale=0.125, accum_out=z2[:])
            rz2 = sbS.tile([128, 1], F32, tag="rz2", name=f"r2_{h}_{t}")
            nc.vector.reciprocal(rz2[:], z2[:])
            nc.vector.tensor_scalar_mul(p2[:, :L], in0=p2[:, :L],
                                        scalar1=rz2[:])
            # strips are stored per i-chunk c (c0: i<512 from jobs t<4,
            # c1: i>=512 from jobs t>=4), tightly packed: [p, (b, i')]
            if t < 4:
                dst = strips_c0[h][:].rearrange("p (b c) -> p b c", c=512)
                dst = dst[:, :t + 1, 128 * t:128 * (t + 1)]
            else:
                dst = strips_c1[h][:].rearrange("p (b c) -> p b c", c=512)
                dst = dst[:, :t + 1, 128 * t - 512:128 * t - 512 + 128]
            nc.sync.dma_start_transpose(dst, p2[:, :L])
            del st[(h, t)]

        pair_pot = {}

        def ot_chunk(h, c):
            g, off = h // 2, 64 * (h % 2)
            i0 = 512 * c
            if h % 2 == 0:
                pair_pot[c] = psot.tile([128, DIM], F32, tag="ot",
                                        name=f"pot{c}_{h}")
            pot = pair_pot[c]
            bs = [b for b in range(NB) if 128 * b < i0 + 512]
            sall = strips_c0[h] if c == 0 else strips_c1[h]
            for b in bs:
                a0 = max(0, 128 * b - i0)
                nc.tensor.matmul(
                    pot[off:off + 64, a0:512],
                    v_sb[b][:, 64 * h:64 * h + 64],
                    sall[:, 512 * b + a0:512 * (b + 1)],
                    start=(b == 0), stop=(b == bs[-1]),
                    tile_position=(0, off))
            if h % 2 == 1:
                gg = h // 2
                nc.gpsimd.tensor_copy(ot_sb[gg][:, i0:i0 + 512], pot[:])

        for hp in range(4):
            h0, h1 = 2 * hp, 2 * hp + 1
            for h in (h0, h1):
                strips_c0[h] = sbStr.tile([128, 4 * 512], BF16,
                                          tag=f"strc0_{h % 2}",
                                          name=f"s0_{h}")
                strips_c1[h] = sbStr.tile([128, NB * 512], BF16,
                                          tag=f"strc1_{h % 2}",
                                          name=f"s1_{h}")
            jobs = []
            for t in range(T):
                jobs.append((h0, t, 0))
                jobs.append((h1, t, 1))

            NJ = len(jobs)
            for n in range(NJ + 4):
                giter = 20 * hp + n
                if n < NJ:
                    stage_exp1(*jobs[n])
                if 1 <= n < NJ + 1:
                    stage_scan(*jobs[n - 1])
                if 2 <= n < NJ + 2:
                    stage_te(*jobs[n - 2])
                if 3 <= n < NJ + 3:
                    stage_s2(*jobs[n - 3])
                if 4 <= n:
                    stage_exp2(*jobs[n - 4])
                popped = 0
                while tail and tail[0][0] <= giter and popped < 3:
                    tail.pop(0)[1]()
                    popped += 1
                if (n == 17 if hp == 0 else n == 11):
                    ot_chunk(h0, 0)
                    ot_chunk(h1, 0)
            ot_chunk(h0, 1)
            ot_chunk(h1, 1)

        # ---- output projection + LayerNorm ----
        for t in range(T):
            psyt = pssB.tile([128, S], F32, tag="sb", name=f"psy{t}")
            psy = psyt[:, :DIM]
            for g in range(4):
                nc.tensor.matmul(psy,
                                 ot_sb[g][:, 128 * t:128 * (t + 1)],
                                 wot[g][:], start=(g == 0), stop=False)
            nc.tensor.matmul(psy, ones1[:], b2row,
                             start=False, stop=True)
            stats = sbS.tile([128, 6], F32, tag="bst")
            nc.vector.bn_stats(out=stats[:], in_=psy)
            mv = sbS.tile([128, 2], F32, tag="bmv")
            nc.vector.bn_aggr(out=mv[:], in_=stats[:])
            rstd = sbS.tile([128, 1], F32, tag="rstd")
            nc.scalar.activation(rstd[:], mv[:, 1:2], AF.Sqrt,
                                 bias=eps_t[:])
            nc.vector.reciprocal(rstd[:], rstd[:])
            y2f = sbA.tile([128, S], F32, tag="expS0", bufs=2,
                           name=f"y2_{t}")
            y2t = y2f[:, :DIM]
            nc.vector.tensor_scalar(out=y2t, in0=psy,
                                    scalar1=mv[:, 0:1], scalar2=rstd[:],
                                    op0=AL.subtract, op1=AL.mult)
            nc.gpsimd.tensor_mul(y2t, y2t, gam_bc[:])
            nc.gpsimd.tensor_add(y2t, y2t, bet_bc[:])
            nc.sync.dma_start(d_out[128 * t:128 * (t + 1), :], y2t)


def kernel(**inputs):
    query = np.asarray(inputs["query"], np.float32)
    key_in = np.asarray(inputs["key_in"], np.float32)
    value = np.asarray(inputs["value"], np.float32)
    B = query.shape[0]
    assert query.shape == (B, S, DIM)

    if "nc" not in _CACHE:
        _CACHE["nc"] = build()
    nc = _CACHE["nc"]

    base = {
        "Wq": np.asarray(inputs["Wq"], np.float32),
        "Wv": np.asarray(inputs["Wv"], np.float32),
        "Wo": np.asarray(inputs["Wo"], np.float32),
        "bq": np.asarray(inputs["bq"], np.float32).reshape(1, DIM),
        "bv": np.asarray(inputs["bv"], np.float32).reshape(1, DIM),
        "bo": np.asarray(inputs["bo"], np.float32).reshape(1, DIM),
        "decay": np.asarray(inputs["decay"], np.float32).reshape(1, H),
        "gamma": np.asarray(inputs["gamma"], np.float32).reshape(1, DIM),
        "beta": np.asarray(inputs["beta"], np.float32).reshape(1, DIM),
    }
    in_maps = []
    for c in range(8):
        b = min(c, B - 1)
        m = dict(base)
        m["query"] = np.ascontiguousarray(query[b])
        m["key_in"] = np.ascontiguousarray(key_in[b])
        m["value"] = np.ascontiguousarray(value[b])
        in_maps.append(m)

    res = bass_utils.run_bass_kernel_spmd(nc, in_maps, core_ids=list(range(8)))
    out = np.stack([res.results[c]["out"] for c in range(B)], 0)
    return out.astype(np.float32)


# revision 39
# speedup vs baseline: 1.0286x; 1.0286x over previous
"""Trainium2 Bass kernel for CRKT layer (decay-reweighted causal attention), v3.

Math per batch b (one NeuronCore per batch element, 8 cores):
  q = query @ Wq.T + bq ; k = key_in @ Wq.T + bq ; v = value @ Wv.T + bv
  s = q k^T  (per head, causal; 1/sqrt(dk) folded into exp scales)
  expS = exp(0.125 s); Z = rowsum; scan = cumsum(expS)
  te = exp((lam/Z) * (scan - Z) * (i-j))     [= exp(-lam*dist)]
  P2 = exp(0.125 * s * te); alpha = P2 / rowsum(P2)
  out = alpha @ v ; y = LN(out @ Wo.T + bo) * gamma + beta

v3 structure (vs v2):
  - ACT (the bottleneck engine) does ONLY the 3 exps per (h,t) job in the
    main loop; all PSUM->SBUF copies moved to Pool, proj biases to DVE.
  - cumsum scan moved DVE -> Pool; stt single-op (scalar_tensor_tensor)
    split Pool (t<6) / DVE (t>=6).
  - causal-mask add matmul in bf16 (1 cyc/row instead of 4 for f32r@128).
  - prologue restructured for early pipeline start: Wq/Q/K loaded first
    (Q/K in halves), only the g=0 projection runs up front; everything
    else (other projections, V/Wo path) issues as paced closures inside
    the attention loop, gated on DMA arrival estimates to avoid engine
    queue head-of-line blocking.
"""

import sys

for _p in ("/opt/trn_rl_repo",):
    if _p not in sys.path:
        sys.path.insert(0, _p)

import numpy as np

import concourse.bass as bass
import concourse.mybir as mybir
import concourse.tile as tile
from concourse import bacc, bass_utils
from concourse.masks import make_identity

F32 = mybir.dt.float32
F32R = mybir.dt.float32r
BF16 = mybir.dt.bfloat16
F16 = mybir.dt.float16
AL = mybir.AluOpType
AF = mybir.ActivationFunctionType

S, DIM, H, DK = 1024, 512, 8, 64
T = S // 128        # 8 i-tiles
NB = S // 128       # 8 j-blocks
NEGBIG = -1e30

_CACHE = {}


def _chunks(total, step):
    return [(a, min(a + step, total)) for a in range(0, total, step)]


def build():
    nc = bacc.Bacc("TRN2", target_bir_lowering=False, debug=False, num_devices=8)

    d_query = nc.dram_tensor("query", [S, DIM], F32, kind="ExternalInput")
    d_key = nc.dram_tensor("key_in", [S, DIM], F32, kind="ExternalInput")
    d_value = nc.dram_tensor("value", [S, DIM], F32, kind="ExternalInput")
    d_wq = nc.dram_tensor("Wq", [DIM, DIM], F32, kind="ExternalInput")
    d_wv = nc.dram_tensor("Wv", [DIM, DIM], F32, kind="ExternalInput")
    d_wo = nc.dram_tensor("Wo", [DIM, DIM], F32, kind="ExternalInput")
    d_bq = nc.dram_tensor("bq", [1, DIM], F32, kind="ExternalInput")
    d_bv = nc.dram_tensor("bv", [1, DIM], F32, kind="ExternalInput")
    d_bo = nc.dram_tensor("bo", [1, DIM], F32, kind="ExternalInput")
    d_dec = nc.dram_tensor("decay", [1, H], F32, kind="ExternalInput")
    d_gam = nc.dram_tensor("gamma", [1, DIM], F32, kind="ExternalInput")
    d_bet = nc.dram_tensor("beta", [1, DIM], F32, kind="ExternalInput")
    d_out = nc.dram_tensor("out", [S, DIM], F32, kind="ExternalOutput")

    with tile.TileContext(nc) as tc:
        _body(nc, tc, d_query, d_key, d_value, d_wq, d_wv, d_wo,
              d_bq, d_bv, d_bo, d_dec, d_gam, d_bet, d_out)

    nc.compile()
    return nc


def _body(nc, tc, d_query, d_key, d_value, d_wq, d_wv, d_wo,
          d_bq, d_bv, d_bo, d_dec, d_gam, d_bet, d_out):
    import contextlib
    ctx = contextlib.ExitStack()
    with ctx:
        const = ctx.enter_context(tc.tile_pool(name="const", bufs=1))
        persist = ctx.enter_context(tc.tile_pool(name="persist", bufs=1))

        # ---- priority DMA loads: big attention-critical tensors first ----
        wa_q = persist.tile([128, 4 * DIM], F32, tag="waq", name="wa_q")
        nc.sync.dma_start(
            wa_q[:].rearrange("p (r d) -> p r d", d=DIM),
            d_wq.ap().rearrange("(r p) d -> p r d", p=128))

        xa_q = persist.tile([128, T * DIM], F32, tag="xaq", name="xa_q")
        xa_k = persist.tile([128, T * DIM], F32, tag="xak", name="xa_k")

        def load_x_half(xa, dram, half):
            t0 = 4 * half
            nc.sync.dma_start(
                xa[:, t0 * DIM:(t0 + 4) * DIM].rearrange(
                    "p (t d) -> p t d", d=DIM),
                dram[128 * 4 * half:128 * 4 * (half + 1), :].rearrange(
                    "(t p) d -> p t d", p=128))

        load_x_half(xa_q, d_query, 0)
        load_x_half(xa_k, d_key, 0)

        # small consts needed early
        lam = const.tile([128, H], F32)     # |decay_h| broadcast down partitions
        nc.sync.dma_start(lam[:], d_dec.ap().to_broadcast((128, H)))
        nc.scalar.activation(lam[:], lam[:], AF.Abs)
        bq_sb = const.tile([128, 4], F32)   # col m = bq[128m:128(m+1)]
        nc.sync.dma_start(bq_sb[:], d_bq[0, :].rearrange("(m p) -> p m", p=128))

        # remaining big loads
        load_x_half(xa_q, d_query, 1)
        load_x_half(xa_k, d_key, 1)

        # late consts
        bv_pk = const.tile([128, 4], F32)   # col r = bv[128r:128(r+1)]
        nc.sync.dma_start(bv_pk[:], d_bv[0, :].rearrange("(m p) -> p m", p=128))
        bv_pkr = const.tile([128, 4], BF16)
        nc.vector.tensor_copy(bv_pkr[:], bv_pk[:])
        bo_rowr = const.tile([1, DIM], F32R)
        nc.gpsimd.dma_start(bo_rowr[:], d_bo[0:1, :])
        gam_bc = const.tile([128, DIM], F32)
        nc.sync.dma_start(gam_bc[:], d_gam.ap().to_broadcast((128, DIM)))
        bet_bc = const.tile([128, DIM], F32)
        nc.sync.dma_start(bet_bc[:], d_bet.ap().to_broadcast((128, DIM)))

        # ---- non-DMA constants ----
        # Single distance tile (Pool, before the first scans):
        # dall[p, c] = 896 + p - c; view for i-tile t is dall[:, 128(7-t):...]
        # so that dall[p, j + 128(7-t)] = 128t + p - j = i - j.
        dall = const.tile([128, S], F16, tag="dall", name="dall")
        nc.gpsimd.iota(dall[:], pattern=[[-1, S]], base=128 * (T - 1),
                       channel_multiplier=1,
                       allow_small_or_imprecise_dtypes=True)

        def d_view(t):
            off = 128 * (T - 1 - t)
            return dall[:, off:off + 128 * (t + 1)]

        ident = const.tile([128, 128], F32)
        make_identity(nc, ident[:])
        identb = const.tile([128, 128], BF16)
        nc.vector.tensor_copy(identb[:], ident[:])
        cmaskb = const.tile([128, 128], BF16)
        nc.gpsimd.memset(cmaskb[:], 0.0)
        nc.gpsimd.affine_select(
            out=cmaskb[:], in_=cmaskb[:], compare_op=AL.is_ge, fill=NEGBIG,
            base=0, channel_multiplier=1, pattern=[[-1, 128]])
        eps_t = const.tile([128, 1], F32)
        nc.vector.memset(eps_t[:], 1e-5)
        ones1 = const.tile([1, 128], F32R)
        nc.vector.memset(ones1[:], 1.0)
        onecol = ones1[:, 0:1]

        # ---- persistent tensors ----
        qt = [persist.tile([128, S], F32R, tag=f"qt{g}", name=f"qt{g}") for g in range(4)]
        kt = [persist.tile([128, S], F32R, tag=f"kt{g}", name=f"kt{g}") for g in range(4)]
        xt_q = [persist.tile([128, S], F32R, tag=f"xtq{d}", name=f"xtq{d}") for d in range(4)]
        xt_k = [persist.tile([128, S], F32R, tag=f"xtk{d}", name=f"xtk{d}") for d in range(4)]
        wqt = [persist.tile([128, DIM], F32R, tag=f"wqt{c}", name=f"wqt{c}") for c in range(4)]
        # The V / Wv / Wo path runs entirely in bf16 via casting gpsimd DMAs
        # + DMA transposes (no PE transposes, no PSUM->SBUF copies).
        # Buffers are allocated inside closures, reusing the xaq/xak tags.
        wvt4 = persist.tile([128, 4 * DIM], BF16, tag="wvt4", name="wvt4")
        wot4 = persist.tile([128, 4 * DIM], BF16, tag="wot4", name="wot4")
        v_sb = [persist.tile([128, DIM], BF16, tag=f"v{t}", name=f"v{t}") for t in range(T)]
        ot_sb = [persist.tile([128, S], BF16, tag=f"ot{g}", name=f"ot{g}") for g in range(4)]
        # Wo^T view for pair g: [p, (r, q)] with j = 128r + q (see _wo_T)
        wot_view = wot4[:].rearrange("p (r c q) -> p c r q", c=4, q=128)
        wot = [wot_view[:, g] for g in range(4)]
        b2row = None  # allocated in _b2, reusing the qt0 tag
        vbf = {}

        # ---- PSUM pools (8 banks total: 2 + 2*2 + 1 = 7 used) ----
        pssA = ctx.enter_context(tc.tile_pool(name="pssA", bufs=1, space="PSUM"))
        pssB = ctx.enter_context(tc.tile_pool(name="pssB", bufs=2, space="PSUM"))
        psot = ctx.enter_context(tc.tile_pool(name="psot", bufs=1, space="PSUM"))
        sbA = ctx.enter_context(tc.tile_pool(name="sbA", bufs=4))
        sbS = ctx.enter_context(tc.tile_pool(name="sbS", bufs=4))
        sbP = ctx.enter_context(tc.tile_pool(name="sbP", bufs=2))
        sbStr = ctx.enter_context(tc.tile_pool(name="sbStr", bufs=1))

        # ---- prologue helpers ----
        def transp_w(wa, wt_dst, c):
            # one column-block c of W^T into wt_dst[c]; copy on DVE
            pt = pssB.tile([128, S], F32, tag="sb", name=f"ptw_{c}")
            for r in range(4):
                nc.tensor.transpose(
                    pt[:, 128 * r:128 * (r + 1)],
                    wa[:, DIM * r + 128 * c:DIM * r + 128 * (c + 1)],
                    ident[:])
            nc.vector.tensor_copy(wt_dst[c][:], pt[:, :DIM])

        def transp_x_half(xa, xt, dblk, half, on_act):
            # transpose 4 t-blocks (one half) of input dblk; copy DVE or ACT
            ptx = pssB.tile([128, S], F32, tag="sb", name=f"ptx_{dblk}_{half}")
            for tt in range(4):
                t = 4 * half + tt
                nc.tensor.transpose(
                    ptx[:, 128 * tt:128 * (tt + 1)],
                    xa[:, DIM * t + 128 * dblk:DIM * t + 128 * (dblk + 1)],
                    ident[:])
            dst = xt[dblk][:, 512 * half:512 * (half + 1)]
            if on_act:
                nc.scalar.activation(dst, ptx[:, :512], AF.Identity)
            else:
                nc.vector.tensor_copy(dst, ptx[:, :512])

        def proj_qk_half(xt, dst_tiles, g, half):
            # qt/kt[g][:, 512h:512h+512] = Wq[g-block] @ x^T[:, half] + bq
            pp = pssB.tile([128, S], F32, tag="sb", name=f"pp_{g}_{half}")
            for kk in range(4):
                nc.tensor.matmul(
                    pp[:, :DIM],
                    wqt[kk][:, 128 * g:128 * (g + 1)],
                    xt[kk][:, DIM * half:DIM * (half + 1)],
                    start=(kk == 0), stop=(kk == 3))
            dst = dst_tiles[g][:, DIM * half:DIM * (half + 1)]
            nc.vector.tensor_scalar_add(dst, in0=pp[:, :DIM],
                                        scalar1=bq_sb[:, g:g + 1])

        # ---- upfront PE work: enough for pair 0, jobs t<4 ----
        for c in range(4):
            transp_w(wa_q, wqt, c)
        for dblk in range(4):
            transp_x_half(xa_q, xt_q, dblk, 0, on_act=False)
        for dblk in range(4):
            transp_x_half(xa_k, xt_k, dblk, 0, on_act=True)
        proj_qk_half(xt_q, qt, 0, 0)
        proj_qk_half(xt_k, kt, 0, 0)

        # ---- paced closures for the rest of the prologue ----
        def _xT1(xa, xt, dblk, on_act):
            def run():
                transp_x_half(xa, xt, dblk, 1, on_act)
            return run

        def _proj(xt, dst, g, half):
            def run():
                proj_qk_half(xt, dst, g, half)
            return run

        # V path in bf16: casting gpsimd DMA load (x_v f32 -> bf16), then a
        # DMA blocked-transpose into the second half of the same buffer.
        # The buffer reuses the xaq tag (xa_q is dead after the xq half-1
        # transposes; the tag ring provides the WAR ordering).
        def _xv_cast():
            vx2 = persist.tile([128, 2 * T * DIM], BF16, tag="xaq",
                               name="vx2")
            vbf["x"] = vx2
            nc.gpsimd.dma_start(
                vx2[:, :T * DIM].rearrange("p (t d) -> p t d", d=DIM),
                d_value.ap().rearrange("(t p) d -> p t d", p=128))

        def _xv_T():
            vx2 = vbf["x"]
            # flat 2D dst in src-tile order (t-major over (t, dblk)):
            # xtv[p, 512*t + 128*dblk + tok] = x_v^T[128*dblk + p, 128*t + tok]
            nc.sync.dma_start_transpose(vx2[:, T * DIM:], vx2[:, :T * DIM])

        def _wv_cast():
            wvb = persist.tile([128, 4 * DIM], BF16, tag="xak", name="wvb")
            vbf["wv"] = wvb
            nc.gpsimd.dma_start(
                wvb[:].rearrange("p (r d) -> p r d", d=DIM),
                d_wv.ap().rearrange("(r p) d -> p r d", p=128))

        def _wv_T():
            # flat 2D dst in src-tile order (r-major over (r, c)):
            # wvt4[p, 512*r + 128*c + q] = Wv[128*r + q, 128*c + p]
            nc.sync.dma_start_transpose(wvt4[:], vbf["wv"][:])

        def _wo_cast():
            wob = persist.tile([128, 4 * DIM], BF16, tag="xak", name="wob")
            vbf["wo"] = wob
            nc.gpsimd.dma_start(
                wob[:].rearrange("p (r d) -> p r d", d=DIM),
                d_wo.ap().rearrange("(r p) d -> p r d", p=128))

        def _wo_T():
            nc.sync.dma_start_transpose(wot4[:], vbf["wo"][:])

        def _vproj(t):
            def run():
                vx2 = vbf["x"]
                # Wv^T view for block kk: [p, (r, q)] with j = 128r + q
                wv_view = wvt4[:].rearrange("p (r c q) -> p c r q", c=4, q=128)
                pp = pssB.tile([128, S], F32, tag="sb", name=f"ppv{t}")
                for kk in range(4):
                    base = T * DIM + 512 * t + 128 * kk
                    nc.tensor.matmul(
                        pp[:, :DIM],
                        vx2[:, base:base + 128],
                        wv_view[:, kk],
                        start=(kk == 0), stop=(kk == 3))
                nc.gpsimd.tensor_copy(v_sb[t][:], pp[:, :DIM])
            return run

        def _b2():
            # b2 = Wo bv + bo (bv folds through PV: alpha rows sum to 1)
            nonlocal b2row
            b2full = persist.tile([128, S], F32R, tag="qt0", name="b2row")
            b2row = b2full[0:1, :DIM]
            b2t = pssB.tile([128, S], F32, tag="sb", name="b2ps")
            b2ps = b2t[0:1, :DIM]
            for g in range(4):
                nc.tensor.matmul(b2ps, bv_pkr[:, g:g + 1],
                                 wot[g], start=(g == 0), stop=False)
            nc.tensor.matmul(b2ps, onecol, bo_rowr[:],
                             start=False, stop=True)
            nc.vector.tensor_copy(b2row, b2ps)

        # (min_global_iter, closure); paced 3/iter when eligible.
        # DMA eta (us): wq 2.9, xq0 5.8, xk0 8.7, xq1 11.6, xk1 14.5, then
        # the bf16 cast-loads for x_v / Wv / Wo (~1.5-3 each).
        # exec-time(iter n) ~= 13 + per-iter work; closures placed so their
        # data has landed by the time the engine queue reaches them.
        # All xt_q readers must be issued before _xv_cast reuses the xaq
        # tag, and all xt_k readers before _wv_cast/_wo_cast reuse xak.
        tail = []
        for d in range(4):
            tail.append((3, _xT1(xa_q, xt_q, d, False)))
        for d in range(4):
            tail.append((5, _xT1(xa_k, xt_k, d, False)))
        tail.append((7, _proj(xt_q, qt, 0, 1)))
        tail.append((7, _proj(xt_k, kt, 0, 1)))
        tail.append((8, _xv_cast))
        for g in range(1, 4):
            for half in range(2):
                tail.append((8 + g, _proj(xt_q, qt, g, half)))
                tail.append((8 + g, _proj(xt_k, kt, g, half)))
        tail.append((9, _wv_cast))
        tail.append((10, _xv_T))
        tail.append((10, _wv_T))
        for t in range(T):
            tail.append((12, _vproj(t)))
        tail.append((13, _wo_cast))
        tail.append((14, _wo_T))
        tail.append((15, _b2))

        # ---- attention ----
        # strided views over packed strips: strips_cX[h][:, b, i'] (b-major)
        strips_c0 = {}
        strips_c1 = {}

        def qk(h, t, ps):
            g, off = h // 2, 64 * (h % 2)
            L = 128 * (t + 1)
            for (j0, j1) in _chunks(L, 512):
                nc.tensor.matmul(
                    ps[:, j0:j1],
                    qt[g][off:off + 64, 128 * t:128 * (t + 1)],
                    kt[g][off:off + 64, j0:j1],
                    start=True, stop=(j1 < L))
            nc.tensor.matmul(ps[:, 128 * t:L], identb[:], cmaskb[:],
                             start=False, stop=True)

        st = {}

        def stage_exp1(h, t, par):
            L = 128 * (t + 1)
            ps = pssA.tile([128, S], F32, tag="sa", name=f"psa_{h}_{t}")
            qk(h, t, ps)
            expS = sbA.tile([128, S], F32, tag=f"expS{par}",
                            name=f"e_{h}_{t}", bufs=2)
            nc.scalar.activation(expS[:, :L], ps[:, :L], AF.Exp,
                                 scale=0.125)
            st[(h, t)] = {"expS": expS}

        def stage_scan(h, t, par):
            L = 128 * (t + 1)
            d = st[(h, t)]
            scan = d["expS"]
            nc.gpsimd.tensor_tensor_scan(
                scan[:, :L], d["expS"][:, :L], d["expS"][:, :L], 0.0,
                op0=AL.add, op1=AL.bypass)
            sc1 = sbS.tile([128, 1], F32, tag="sc1", name=f"s1_{h}_{t}")
            z1 = scan[:, L - 1:L]
            nc.vector.reciprocal(sc1[:], z1)
            nc.vector.tensor_scalar_mul(sc1[:], in0=sc1[:],
                                        scalar1=lam[:, h:h + 1])
            d["sc1"] = sc1
            # stt: u = (scan - Z) * d (in place), single op
            eng = nc.gpsimd if t < 6 else nc.vector
            eng.scalar_tensor_tensor(
                scan[:, :L], in0=scan[:, :L], scalar=z1,
                in1=d_view(t), op0=AL.subtract, op1=AL.mult)

        def stage_te(h, t, par):
            L = 128 * (t + 1)
            d = st[(h, t)]
            te = sbP.tile([128, S], BF16, tag=f"te{par}",
                          name=f"te_{h}_{t}", bufs=1)
            nc.scalar.activation(te[:, :L], d["expS"][:, :L], AF.Exp,
                                 scale=d["sc1"][:])
            d["te"] = te

        def stage_s2(h, t, par):
            L = 128 * (t + 1)
            d = st[(h, t)]
            ps = pssB.tile([128, S], F32, tag="sb", name=f"psb_{h}_{t}")
            qk(h, t, ps)
            nc.vector.tensor_mul(ps[:, :L], ps[:, :L], d["te"][:, :L])
            d["ps"] = ps

        def stage_exp2(h, t, par):
            L = 128 * (t + 1)
            d = st[(h, t)]
            p2 = sbA.tile([128, S], BF16, tag=f"p2{par}",
                          name=f"p2_{h}_{t}", bufs=1)
            z2 = sbS.tile([128, 1], F32, tag="z2", name=f"z2_{h}_{t}")
            nc.scalar.activation(p2[:, :L], d["ps"][:, :L], AF.Exp,
                                 scale=0.125, accum_out=z2[:])
            rz2 = sbS.tile([128, 1], F32, tag="rz2", name=f"r2_{h}_{t}")
            nc.vector.reciprocal(rz2[:], z2[:])
            nc.vector.tensor_scalar_mul(p2[:, :L], in0=p2[:, :L],
                                        scalar1=rz2[:])
            # strips are stored per i-chunk c (c0: i<512 from jobs t<4,
            # c1: i>=512 from jobs t>=4), tightly packed: [p, (b, i')]
            if t < 4:
                dst = strips_c0[h][:].rearrange("p (b c) -> p b c", c=512)
                dst = dst[:, :t + 1, 128 * t:128 * (t + 1)]
            else:
                dst = strips_c1[h][:].rearrange("p (b c) -> p b c", c=512)
                dst = dst[:, :t + 1, 128 * t - 512:128 * t - 512 + 128]
            nc.sync.dma_start_transpose(dst, p2[:, :L])
            del st[(h, t)]

        pair_pot = {}

        def ot_chunk(h, c):
            g, off = h // 2, 64 * (h % 2)
            i0 = 512 * c
            if h % 2 == 0:
                pair_pot[c] = psot.tile([128, DIM], F32, tag="ot",
                                        name=f"pot{c}_{h}")
            pot = pair_pot[c]
            bs = [b for b in range(NB) if 128 * b < i0 + 512]
            sall = strips_c0[h] if c == 0 else strips_c1[h]
            for b in bs:
                a0 = max(0, 128 * b - i0)
                nc.tensor.matmul(
                    pot[off:off + 64, a0:512],
                    v_sb[b][:, 64 * h:64 * h + 64],
                    sall[:, 512 * b + a0:512 * (b + 1)],
                    start=(b == 0), stop=(b == bs[-1]),
                    tile_position=(0, off))
            if h % 2 == 1:
                gg = h // 2
                nc.gpsimd.tensor_copy(ot_sb[gg][:, i0:i0 + 512], pot[:])

        for hp in range(4):
            h0, h1 = 2 * hp, 2 * hp + 1
            for h in (h0, h1):
                strips_c0[h] = sbStr.tile([128, 4 * 512], BF16,
                                          tag=f"strc0_{h % 2}",
                                          name=f"s0_{h}")
                strips_c1[h] = sbStr.tile([128, NB * 512], BF16,
                                          tag=f"strc1_{h % 2}",
                                          name=f"s1_{h}")
            jobs = []
            for t in range(T):
                jobs.append((h0, t, 0))
                jobs.append((h1, t, 1))

            NJ = len(jobs)
            for n in range(NJ + 4):
                giter = 20 * hp + n
                if n < NJ:
                    stage_exp1(*jobs[n])
                if 1 <= n < NJ + 1:
                    stage_scan(*jobs[n - 1])
                if 2 <= n < NJ + 2:
                    stage_te(*jobs[n - 2])
                if 3 <= n < NJ + 3:
                    stage_s2(*jobs[n - 3])
                if 4 <= n:
                    stage_exp2(*jobs[n - 4])
                popped = 0
                while tail and tail[0][0] <= giter and popped < 3:
                    tail.pop(0)[1]()
                    popped += 1
                if (n == 17 if hp == 0 else n == 11):
                    ot_chunk(h0, 0)
                    ot_chunk(h1, 0)
            ot_chunk(h0, 1)
            ot_chunk(h1, 1)

        # ---- output projection + LayerNorm ----
        for t in range(T):
            psyt = pssB.tile([128, S], F32, tag="sb", name=f"psy{t}")
            psy = psyt[:, :DIM]
            for g in range(4):
                nc.tensor.matmul(psy,
                                 ot_sb[g][:, 128 * t:128 * (t + 1)],
                                 wot[g], start=(g == 0), stop=False)
            nc.tensor.matmul(psy, ones1[:], b2row,
                             start=False, stop=True)
            stats = sbS.tile([128, 6], F32, tag="bst")
            nc.vector.bn_stats(out=stats[:], in_=psy)
            mv = sbS.tile([128, 2], F32, tag="bmv")
            nc.vector.bn_aggr(out=mv[:], in_=stats[:])
            rstd = sbS.tile([128, 1], F32, tag="rstd")
            nc.scalar.activation(rstd[:], mv[:, 1:2], AF.Sqrt,
                                 bias=eps_t[:])
            nc.vector.reciprocal(rstd[:], rstd[:])
            y2f = sbA.tile([128, S], F32, tag="expS0", bufs=2,
                           name=f"y2_{t}")
            y2t = y2f[:, :DIM]
            nc.vector.tensor_scalar(out=y2t, in0=psy,
                                    scalar1=mv[:, 0:1], scalar2=rstd[:],
                                    op0=AL.subtract, op1=AL.mult)
            nc.gpsimd.tensor_mul(y2t, y2t, gam_bc[:])
            nc.gpsimd.tensor_add(y2t, y2t, bet_bc[:])
            nc.sync.dma_start(d_out[128 * t:128 * (t + 1), :], y2t)


def kernel(**inputs):
    query = np.asarray(inputs["query"], np.float32)
    key_in = np.asarray(inputs["key_in"], np.float32)
    value = np.asarray(inputs["value"], np.float32)
    B = query.shape[0]
    assert query.shape == (B, S, DIM)

    if "nc" not in _CACHE:
        _CACHE["nc"] = build()
    nc = _CACHE["nc"]

    base = {
        "Wq": np.asarray(inputs["Wq"], np.float32),
        "Wv": np.asarray(inputs["Wv"], np.float32),
        "Wo": np.asarray(inputs["Wo"], np.float32),
        "bq": np.asarray(inputs["bq"], np.float32).reshape(1, DIM),
        "bv": np.asarray(inputs["bv"], np.float32).reshape(1, DIM),
        "bo": np.asarray(inputs["bo"], np.float32).reshape(1, DIM),
        "decay": np.asarray(inputs["decay"], np.float32).reshape(1, H),
        "gamma": np.asarray(inputs["gamma"], np.float32).reshape(1, DIM),
        "beta": np.asarray(inputs["beta"], np.float32).reshape(1, DIM),
    }
    in_maps = []
    for c in range(8):
        b = min(c, B - 1)
        m = dict(base)
        m["query"] = np.ascontiguousarray(query[b])
        m["key_in"] = np.ascontiguousarray(key_in[b])
        m["value"] = np.ascontiguousarray(value[b])
        in_maps.append(m)

    res = bass_utils.run_bass_kernel_spmd(nc, in_maps, core_ids=list(range(8)))
    out = np.stack([res.results[c]["out"] for c in range(B)], 0)
    return out.astype(np.float32)


# revision 44
# speedup vs baseline: 1.1301x; 1.0987x over previous
"""Trainium2 Bass kernel for CRKT layer (decay-reweighted causal attention), v3.

Math per batch b (one NeuronCore per batch element, 8 cores):
  q = query @ Wq.T + bq ; k = key_in @ Wq.T + bq ; v = value @ Wv.T + bv
  s = q k^T  (per head, causal; 1/sqrt(dk) folded into exp scales)
  expS = exp(0.125 s); Z = rowsum; scan = cumsum(expS)
  te = exp((lam/Z) * (scan - Z) * (i-j))     [= exp(-lam*dist)]
  P2 = exp(0.125 * s * te); alpha = P2 / rowsum(P2)
  out = alpha @ v ; y = LN(out @ Wo.T + bo) * gamma + beta

v3 structure (vs v2):
  - ACT (the bottleneck engine) does ONLY the 3 exps per (h,t) job in the
    main loop; all PSUM->SBUF copies moved to Pool, proj biases to DVE.
  - cumsum scan moved DVE -> Pool; stt single-op (scalar_tensor_tensor)
    split Pool (t<6) / DVE (t>=6).
  - causal-mask add matmul in bf16 (1 cyc/row instead of 4 for f32r@128).
  - prologue restructured for early pipeline start: Wq/Q/K loaded first
    (Q/K in halves), only the g=0 projection runs up front; everything
    else (other projections, V/Wo path) issues as paced closures inside
    the attention loop, gated on DMA arrival estimates to avoid engine
    queue head-of-line blocking.
"""

import sys

for _p in ("/opt/trn_rl_repo",):
    if _p not in sys.path:
        sys.path.insert(0, _p)

import numpy as np

import concourse.bass as bass
import concourse.mybir as mybir
import concourse.tile as tile
from concourse import bacc, bass_utils
from concourse.masks import make_identity

F32 = mybir.dt.float32
F32R = mybir.dt.float32r
BF16 = mybir.dt.bfloat16
F16 = mybir.dt.float16
AL = mybir.AluOpType
AF = mybir.ActivationFunctionType

S, DIM, H, DK = 1024, 512, 8, 64
T = S // 128        # 8 i-tiles
NB = S // 128       # 8 j-blocks
NEGBIG = -1e30

_CACHE = {}


def _chunks(total, step):
    return [(a, min(a + step, total)) for a in range(0, total, step)]


def build():
    nc = bacc.Bacc("TRN2", target_bir_lowering=False, debug=False, num_devices=8)

    d_query = nc.dram_tensor("query", [S, DIM], F32, kind="ExternalInput")
    d_key = nc.dram_tensor("key_in", [S, DIM], F32, kind="ExternalInput")
    d_value = nc.dram_tensor("value", [S, DIM], F32, kind="ExternalInput")
    d_wq = nc.dram_tensor("Wq", [DIM, DIM], F32, kind="ExternalInput")
    d_wv = nc.dram_tensor("Wv", [DIM, DIM], F32, kind="ExternalInput")
    d_wo = nc.dram_tensor("Wo", [DIM, DIM], F32, kind="ExternalInput")
    d_bq = nc.dram_tensor("bq", [1, DIM], F32, kind="ExternalInput")
    d_bv = nc.dram_tensor("bv", [1, DIM], F32, kind="ExternalInput")
    d_bo = nc.dram_tensor("bo", [1, DIM], F32, kind="ExternalInput")
    d_dec = nc.dram_tensor("decay", [1, H], F32, kind="ExternalInput")
    d_gam = nc.dram_tensor("gamma", [1, DIM], F32, kind="ExternalInput")
    d_bet = nc.dram_tensor("beta", [1, DIM], F32, kind="ExternalInput")
    d_out = nc.dram_tensor("out", [S, DIM], F32, kind="ExternalOutput")

    with tile.TileContext(nc) as tc:
        _body(nc, tc, d_query, d_key, d_value, d_wq, d_wv, d_wo,
              d_bq, d_bv, d_bo, d_dec, d_gam, d_bet, d_out)

    nc.compile()
    return nc


def _body(nc, tc, d_query, d_key, d_value, d_wq, d_wv, d_wo,
          d_bq, d_bv, d_bo, d_dec, d_gam, d_bet, d_out):
    import contextlib
    ctx = contextlib.ExitStack()
    with ctx:
        const = ctx.enter_context(tc.tile_pool(name="const", bufs=1))
        persist = ctx.enter_context(tc.tile_pool(name="persist", bufs=1))

        # ---- priority DMA loads: big attention-critical tensors first ----
        wa_q = persist.tile([128, 4 * DIM], F32, tag="waq", name="wa_q")
        nc.sync.dma_start(
            wa_q[:].rearrange("p (r d) -> p r d", d=DIM),
            d_wq.ap().rearrange("(r p) d -> p r d", p=128))

        xa_q = persist.tile([128, T * DIM], F32, tag="xaq", name="xa_q")
        xa_k = persist.tile([128, T * DIM], F32, tag="xak", name="xa_k")

        def load_x_half(xa, dram, half):
            t0 = 4 * half
            nc.sync.dma_start(
                xa[:, t0 * DIM:(t0 + 4) * DIM].rearrange(
                    "p (t d) -> p t d", d=DIM),
                dram[128 * 4 * half:128 * 4 * (half + 1), :].rearrange(
                    "(t p) d -> p t d", p=128))

        load_x_half(xa_q, d_query, 0)
        load_x_half(xa_k, d_key, 0)

        # small consts needed early
        lam = const.tile([128, H], F32)     # |decay_h| broadcast down partitions
        nc.sync.dma_start(lam[:], d_dec.ap().to_broadcast((128, H)))
        nc.scalar.activation(lam[:], lam[:], AF.Abs)
        bq_sb = const.tile([128, 4], F32)   # col m = bq[128m:128(m+1)]
        nc.sync.dma_start(bq_sb[:], d_bq[0, :].rearrange("(m p) -> p m", p=128))

        # remaining big loads
        load_x_half(xa_q, d_query, 1)
        load_x_half(xa_k, d_key, 1)

        # late consts
        bv_pk = const.tile([128, 4], F32)   # col r = bv[128r:128(r+1)]
        nc.sync.dma_start(bv_pk[:], d_bv[0, :].rearrange("(m p) -> p m", p=128))
        bv_pkr = const.tile([128, 4], BF16)
        nc.vector.tensor_copy(bv_pkr[:], bv_pk[:])
        bo_rowr = const.tile([1, DIM], F32R)
        nc.gpsimd.dma_start(bo_rowr[:], d_bo[0:1, :])
        gam_bc = const.tile([128, DIM], F32)
        nc.sync.dma_start(gam_bc[:], d_gam.ap().to_broadcast((128, DIM)))
        bet_bc = const.tile([128, DIM], F32)
        nc.sync.dma_start(bet_bc[:], d_bet.ap().to_broadcast((128, DIM)))

        # ---- non-DMA constants ----
        # Single distance tile (Pool, before the first scans):
        # dall[p, c] = 896 + p - c; view for i-tile t is dall[:, 128(7-t):...]
        # so that dall[p, j + 128(7-t)] = 128t + p - j = i - j.
        dall = const.tile([128, S], F16, tag="dall", name="dall")
        nc.gpsimd.iota(dall[:], pattern=[[-1, S]], base=128 * (T - 1),
                       channel_multiplier=1,
                       allow_small_or_imprecise_dtypes=True)

        def d_view(t):
            off = 128 * (T - 1 - t)
            return dall[:, off:off + 128 * (t + 1)]

        ident = const.tile([128, 128], F32)
        make_identity(nc, ident[:])
        identb = const.tile([128, 128], BF16)
        nc.vector.tensor_copy(identb[:], ident[:])
        cmaskb = const.tile([128, 128], BF16)
        nc.gpsimd.memset(cmaskb[:], 0.0)
        nc.gpsimd.affine_select(
            out=cmaskb[:], in_=cmaskb[:], compare_op=AL.is_ge, fill=NEGBIG,
            base=0, channel_multiplier=1, pattern=[[-1, 128]])
        eps_t = const.tile([128, 1], F32)
        nc.vector.memset(eps_t[:], 1e-5)
        ones1 = const.tile([1, 128], F32R)
        nc.vector.memset(ones1[:], 1.0)
        onecol = ones1[:, 0:1]

        # ---- persistent tensors ----
        qt = [persist.tile([128, S], F32R, tag=f"qt{g}", name=f"qt{g}") for g in range(4)]
        kt = [persist.tile([128, S], F32R, tag=f"kt{g}", name=f"kt{g}") for g in range(4)]
        xt_q = [persist.tile([128, S], F32R, tag=f"xtq{d}", name=f"xtq{d}") for d in range(4)]
        xt_k = [persist.tile([128, S], F32R, tag=f"xtk{d}", name=f"xtk{d}") for d in range(4)]
        wqt = [persist.tile([128, DIM], F32R, tag=f"wqt{c}", name=f"wqt{c}") for c in range(4)]
        # The V / Wv / Wo path runs entirely in bf16 via casting gpsimd DMAs
        # + DMA transposes (no PE transposes, no PSUM->SBUF copies).
        # Buffers are allocated inside closures, reusing the xaq/xak tags.
        wvt4 = persist.tile([128, 4 * DIM], BF16, tag="wvt4", name="wvt4")
        wot4 = persist.tile([128, 4 * DIM], BF16, tag="wot4", name="wot4")
        v_sb = [persist.tile([128, DIM], BF16, tag=f"v{t}", name=f"v{t}") for t in range(T)]
        ot_sb = [persist.tile([128, S], BF16, tag=f"ot{g}", name=f"ot{g}") for g in range(4)]
        # Wo^T view for pair g: [p, (r, q)] with j = 128r + q (see _wo_T)
        wot_view = wot4[:].rearrange("p (r c q) -> p c r q", c=4, q=128)
        wot = [wot_view[:, g] for g in range(4)]
        b2row = None  # allocated in _b2, reusing the qt0 tag
        vbf = {}

        # ---- PSUM pools (8 banks total: 2 + 2*2 + 1 = 7 used) ----
        pssA = ctx.enter_context(tc.tile_pool(name="pssA", bufs=1, space="PSUM"))
        pssB = ctx.enter_context(tc.tile_pool(name="pssB", bufs=2, space="PSUM"))
        psot = ctx.enter_context(tc.tile_pool(name="psot", bufs=1, space="PSUM"))
        sbA = ctx.enter_context(tc.tile_pool(name="sbA", bufs=4))
        sbS = ctx.enter_context(tc.tile_pool(name="sbS", bufs=4))
        sbP = ctx.enter_context(tc.tile_pool(name="sbP", bufs=2))
        sbStr = ctx.enter_context(tc.tile_pool(name="sbStr", bufs=1))

        # ---- prologue helpers ----
        def transp_w(wa, wt_dst, c):
            # one column-block c of W^T into wt_dst[c]; copy on DVE
            pt = pssB.tile([128, S], F32, tag="sb", name=f"ptw_{c}")
            for r in range(4):
                nc.tensor.transpose(
                    pt[:, 128 * r:128 * (r + 1)],
                    wa[:, DIM * r + 128 * c:DIM * r + 128 * (c + 1)],
                    ident[:])
            nc.vector.tensor_copy(wt_dst[c][:], pt[:, :DIM])

        def transp_x_half(xa, xt, dblk, half, on_act):
            # transpose 4 t-blocks (one half) of input dblk; copy DVE or ACT
            ptx = pssB.tile([128, S], F32, tag="sb", name=f"ptx_{dblk}_{half}")
            for tt in range(4):
                t = 4 * half + tt
                nc.tensor.transpose(
                    ptx[:, 128 * tt:128 * (tt + 1)],
                    xa[:, DIM * t + 128 * dblk:DIM * t + 128 * (dblk + 1)],
                    ident[:])
            dst = xt[dblk][:, 512 * half:512 * (half + 1)]
            if on_act:
                nc.scalar.activation(dst, ptx[:, :512], AF.Identity)
            else:
                nc.vector.tensor_copy(dst, ptx[:, :512])

        def proj_qk_half(xt, dst_tiles, g, half):
            # qt/kt[g][:, 512h:512h+512] = Wq[g-block] @ x^T[:, half] + bq
            pp = pssB.tile([128, S], F32, tag="sb", name=f"pp_{g}_{half}")
            for kk in range(4):
                nc.tensor.matmul(
                    pp[:, :DIM],
                    wqt[kk][:, 128 * g:128 * (g + 1)],
                    xt[kk][:, DIM * half:DIM * (half + 1)],
                    start=(kk == 0), stop=(kk == 3))
            dst = dst_tiles[g][:, DIM * half:DIM * (half + 1)]
            nc.vector.tensor_scalar_add(dst, in0=pp[:, :DIM],
                                        scalar1=bq_sb[:, g:g + 1])

        # ---- upfront PE work: enough for pair 0, jobs t<4 ----
        for c in range(4):
            transp_w(wa_q, wqt, c)
        for dblk in range(4):
            transp_x_half(xa_q, xt_q, dblk, 0, on_act=False)
        for dblk in range(4):
            transp_x_half(xa_k, xt_k, dblk, 0, on_act=True)
        proj_qk_half(xt_q, qt, 0, 0)
        proj_qk_half(xt_k, kt, 0, 0)

        # ---- paced closures for the rest of the prologue ----
        def _xT1(xa, xt, dblk, on_act):
            def run():
                transp_x_half(xa, xt, dblk, 1, on_act)
            return run

        def _proj(xt, dst, g, half):
            def run():
                proj_qk_half(xt, dst, g, half)
            return run

        # V path in bf16: casting gpsimd DMA load (x_v f32 -> bf16), then a
        # DMA blocked-transpose into the second half of the same buffer.
        # The buffer reuses the xaq tag (xa_q is dead after the xq half-1
        # transposes; the tag ring provides the WAR ordering).
        def _xv_cast():
            vx2 = persist.tile([128, 2 * T * DIM], BF16, tag="xaq",
                               name="vx2")
            vbf["x"] = vx2
            nc.gpsimd.dma_start(
                vx2[:, :T * DIM].rearrange("p (t d) -> p t d", d=DIM),
                d_value.ap().rearrange("(t p) d -> p t d", p=128))

        def _xv_T():
            vx2 = vbf["x"]
            # flat 2D dst in src-tile order (t-major over (t, dblk)):
            # xtv[p, 512*t + 128*dblk + tok] = x_v^T[128*dblk + p, 128*t + tok]
            nc.sync.dma_start_transpose(vx2[:, T * DIM:], vx2[:, :T * DIM])

        def _wv_cast():
            wvb = persist.tile([128, 4 * DIM], BF16, tag="xak", name="wvb")
            vbf["wv"] = wvb
            nc.gpsimd.dma_start(
                wvb[:].rearrange("p (r d) -> p r d", d=DIM),
                d_wv.ap().rearrange("(r p) d -> p r d", p=128))

        def _wv_T():
            # flat 2D dst in src-tile order (r-major over (r, c)):
            # wvt4[p, 512*r + 128*c + q] = Wv[128*r + q, 128*c + p]
            nc.sync.dma_start_transpose(wvt4[:], vbf["wv"][:])

        def _wo_cast():
            wob = persist.tile([128, 4 * DIM], BF16, tag="xak", name="wob")
            vbf["wo"] = wob
            nc.gpsimd.dma_start(
                wob[:].rearrange("p (r d) -> p r d", d=DIM),
                d_wo.ap().rearrange("(r p) d -> p r d", p=128))

        def _wo_T():
            nc.sync.dma_start_transpose(wot4[:], vbf["wo"][:])

        def _vproj(t):
            def run():
                vx2 = vbf["x"]
                # Wv^T view for block kk: [p, (r, q)] with j = 128r + q
                wv_view = wvt4[:].rearrange("p (r c q) -> p c r q", c=4, q=128)
                pp = pssB.tile([128, S], F32, tag="sb", name=f"ppv{t}")
                for kk in range(4):
                    base = T * DIM + 512 * t + 128 * kk
                    nc.tensor.matmul(
                        pp[:, :DIM],
                        vx2[:, base:base + 128],
                        wv_view[:, kk],
                        start=(kk == 0), stop=(kk == 3))
                nc.gpsimd.tensor_copy(v_sb[t][:], pp[:, :DIM])
            return run

        def _b2():
            # b2 = Wo bv + bo (bv folds through PV: alpha rows sum to 1)
            nonlocal b2row
            b2full = persist.tile([128, S], F32R, tag="qt0", name="b2row")
            b2row = b2full[0:1, :DIM]
            b2t = pssB.tile([128, S], F32, tag="sb", name="b2ps")
            b2ps = b2t[0:1, :DIM]
            for g in range(4):
                nc.tensor.matmul(b2ps, bv_pkr[:, g:g + 1],
                                 wot[g], start=(g == 0), stop=False)
            nc.tensor.matmul(b2ps, onecol, bo_rowr[:],
                             start=False, stop=True)
            nc.vector.tensor_copy(b2row, b2ps)

        # (min_global_iter, closure); paced 3/iter when eligible.
        # DMA eta (us): wq 2.9, xq0 5.8, xk0 8.7, xq1 11.6, xk1 14.5, then
        # the bf16 cast-loads for x_v / Wv / Wo (~1.5-3 each).
        # exec-time(iter n) ~= 13 + per-iter work; closures placed so their
        # data has landed by the time the engine queue reaches them.
        # All xt_q readers must be issued before _xv_cast reuses the xaq
        # tag, and all xt_k readers before _wv_cast/_wo_cast reuse xak.
        tail = []
        for d in range(4):
            tail.append((3, _xT1(xa_q, xt_q, d, False)))
        for d in range(4):
            tail.append((5, _xT1(xa_k, xt_k, d, False)))
        tail.append((7, _proj(xt_q, qt, 0, 1)))
        tail.append((7, _proj(xt_k, kt, 0, 1)))
        tail.append((8, _xv_cast))
        for g in range(1, 4):
            for half in range(2):
                tail.append((8 + g, _proj(xt_q, qt, g, half)))
                tail.append((8 + g, _proj(xt_k, kt, g, half)))
        tail.append((9, _wv_cast))
        tail.append((10, _xv_T))
        tail.append((10, _wv_T))
        for t in range(T):
            tail.append((12, _vproj(t)))
        tail.append((13, _wo_cast))
        tail.append((14, _wo_T))
        tail.append((15, _b2))

        # ---- attention ----
        # strided views over packed strips: strips_cX[h][:, b, i'] (b-major)
        strips_c0 = {}
        strips_c1 = {}

        def qk(h, t, ps):
            g, off = h // 2, 64 * (h % 2)
            L = 128 * (t + 1)
            for (j0, j1) in _chunks(L, 512):
                nc.tensor.matmul(
                    ps[:, j0:j1],
                    qt[g][off:off + 64, 128 * t:128 * (t + 1)],
                    kt[g][off:off + 64, j0:j1],
                    start=True, stop=(j1 < L))
            nc.tensor.matmul(ps[:, 128 * t:L], identb[:], cmaskb[:],
                             start=False, stop=True)

        st = {}

        def stage_exp1(h, t, par):
            L = 128 * (t + 1)
            ps = pssA.tile([128, S], F32, tag="sa", name=f"psa_{h}_{t}")
            qk(h, t, ps)
            expS = sbA.tile([128, S], F32, tag=f"expS{par}",
                            name=f"e_{h}_{t}", bufs=2)
            nc.scalar.activation(expS[:, :L], ps[:, :L], AF.Exp,
                                 scale=0.125)
            st[(h, t)] = {"expS": expS}

        def stage_scan(h, t, par):
            L = 128 * (t + 1)
            d = st[(h, t)]
            scan = d["expS"]
            nc.gpsimd.tensor_tensor_scan(
                scan[:, :L], d["expS"][:, :L], d["expS"][:, :L], 0.0,
                op0=AL.add, op1=AL.bypass)
            sc1 = sbS.tile([128, 1], F32, tag="sc1", name=f"s1_{h}_{t}")
            z1 = scan[:, L - 1:L]
            nc.vector.reciprocal(sc1[:], z1)
            nc.vector.tensor_scalar_mul(sc1[:], in0=sc1[:],
                                        scalar1=lam[:, h:h + 1])
            d["sc1"] = sc1
            # stt: u = (scan - Z) * d (in place), single op
            eng = nc.gpsimd if t < 6 else nc.vector
            eng.scalar_tensor_tensor(
                scan[:, :L], in0=scan[:, :L], scalar=z1,
                in1=d_view(t), op0=AL.subtract, op1=AL.mult)

        def stage_te(h, t, par):
            L = 128 * (t + 1)
            d = st[(h, t)]
            te = sbP.tile([128, S], BF16, tag=f"te{par}",
                          name=f"te_{h}_{t}", bufs=2)
            nc.scalar.activation(te[:, :L], d["expS"][:, :L], AF.Exp,
                                 scale=d["sc1"][:])
            d["te"] = te

        def stage_s2(h, t, par):
            L = 128 * (t + 1)
            d = st[(h, t)]
            ps = pssB.tile([128, S], F32, tag="sb", name=f"psb_{h}_{t}")
            qk(h, t, ps)
            nc.vector.tensor_mul(ps[:, :L], ps[:, :L], d["te"][:, :L])
            d["ps"] = ps

        def stage_exp2(h, t, par):
            L = 128 * (t + 1)
            d = st[(h, t)]
            if h not in strips_c0:
                strips_c0[h] = sbStr.tile([128, 4 * 512], BF16,
                                          tag=f"strc0_{h % 2}",
                                          name=f"s0_{h}")
                strips_c1[h] = sbStr.tile([128, NB * 512], BF16,
                                          tag=f"strc1_{h % 2}",
                                          name=f"s1_{h}")
            p2 = sbA.tile([128, S], BF16, tag=f"p2{par}",
                          name=f"p2_{h}_{t}", bufs=2)
            z2 = sbS.tile([128, 1], F32, tag="z2", name=f"z2_{h}_{t}")
            nc.scalar.activation(p2[:, :L], d["ps"][:, :L], AF.Exp,
                                 scale=0.125, accum_out=z2[:])
            rz2 = sbS.tile([128, 1], F32, tag="rz2", name=f"r2_{h}_{t}")
            nc.vector.reciprocal(rz2[:], z2[:])
            nc.vector.tensor_scalar_mul(p2[:, :L], in0=p2[:, :L],
                                        scalar1=rz2[:])
            # strips are stored per i-chunk c (c0: i<512 from jobs t<4,
            # c1: i>=512 from jobs t>=4), tightly packed: [p, (b, i')]
            if t < 4:
                dst = strips_c0[h][:].rearrange("p (b c) -> p b c", c=512)
                dst = dst[:, :t + 1, 128 * t:128 * (t + 1)]
            else:
                dst = strips_c1[h][:].rearrange("p (b c) -> p b c", c=512)
                dst = dst[:, :t + 1, 128 * t - 512:128 * t - 512 + 128]
            nc.sync.dma_start_transpose(dst, p2[:, :L])
            del st[(h, t)]

        pair_pot = {}

        def ot_chunk(h, c):
            g, off = h // 2, 64 * (h % 2)
            i0 = 512 * c
            if h % 2 == 0:
                pair_pot[c] = psot.tile([128, DIM], F32, tag="ot",
                                        name=f"pot{c}_{h}")
            pot = pair_pot[c]
            bs = [b for b in range(NB) if 128 * b < i0 + 512]
            sall = strips_c0[h] if c == 0 else strips_c1[h]
            for b in bs:
                a0 = max(0, 128 * b - i0)
                nc.tensor.matmul(
                    pot[off:off + 64, a0:512],
                    v_sb[b][:, 64 * h:64 * h + 64],
                    sall[:, 512 * b + a0:512 * (b + 1)],
                    start=(b == 0), stop=(b == bs[-1]),
                    tile_position=(0, off))
            if h % 2 == 1:
                gg = h // 2
                nc.gpsimd.tensor_copy(ot_sb[gg][:, i0:i0 + 512], pot[:])

        # One flat 64-job pipeline (no per-pair drains). t-order interleaves
        # small and large jobs so large jobs' engine time hides the small
        # jobs' cross-engine latency chains.
        # pair 0 must stay ascending: its t>=4 jobs depend on the half-1
        # projections that are issued as closures around iter 7.
        jobs = []
        for hp in range(4):
            tord = range(T) if hp == 0 else (0, 4, 1, 5, 2, 6, 3, 7)
            for t in tord:
                jobs.append((2 * hp, t, 0))
                jobs.append((2 * hp + 1, t, 1))

        NJ = len(jobs)
        for n in range(NJ + 4):
            if n < NJ:
                stage_exp1(*jobs[n])
            if 1 <= n < NJ + 1:
                stage_scan(*jobs[n - 1])
            if 2 <= n < NJ + 2:
                stage_te(*jobs[n - 2])
            if 3 <= n < NJ + 3:
                stage_s2(*jobs[n - 3])
            if 4 <= n:
                stage_exp2(*jobs[n - 4])
            popped = 0
            while tail and tail[0][0] <= n and popped < 3:
                tail.pop(0)[1]()
                popped += 1
            for hp in range(4):
                # c0 strips (both heads, t<4) complete at D of job 13+4;
                # all strips at D of job 15+4.
                if n == 16 * hp + 17:
                    ot_chunk(2 * hp, 0)
                    ot_chunk(2 * hp + 1, 0)
                if n == 16 * hp + 19:
                    ot_chunk(2 * hp, 1)
                    ot_chunk(2 * hp + 1, 1)

        # ---- output projection + LayerNorm (2-deep software pipeline) ----
        def psy_mm(t):
            psyt = pssB.tile([128, S], F32, tag="sb", name=f"psy{t}")
            psy = psyt[:, :DIM]
            for g in range(4):
                nc.tensor.matmul(psy,
                                 ot_sb[g][:, 128 * t:128 * (t + 1)],
                                 wot[g], start=(g == 0), stop=False)
            nc.tensor.matmul(psy, ones1[:], b2row,
                             start=False, stop=True)
            return psy

        def ln_chain(t, psy):
            stats = sbS.tile([128, 6], F32, tag="bst")
            nc.vector.bn_stats(out=stats[:], in_=psy)
            mv = sbS.tile([128, 2], F32, tag="bmv")
            nc.vector.bn_aggr(out=mv[:], in_=stats[:])
            rstd = sbS.tile([128, 1], F32, tag="rstd")
            nc.scalar.activation(rstd[:], mv[:, 1:2], AF.Sqrt,
                                 bias=eps_t[:])
            nc.vector.reciprocal(rstd[:], rstd[:])
            y2f = sbA.tile([128, S], F32, tag="expS0", bufs=2,
                           name=f"y2_{t}")
            y2t = y2f[:, :DIM]
            nc.vector.tensor_scalar(out=y2t, in0=psy,
                                    scalar1=mv[:, 0:1], scalar2=rstd[:],
                                    op0=AL.subtract, op1=AL.mult)
            nc.vector.tensor_mul(y2t, y2t, gam_bc[:])
            nc.vector.tensor_add(y2t, y2t, bet_bc[:])
            nc.sync.dma_start(d_out[128 * t:128 * (t + 1), :], y2t)

        psy_prev = psy_mm(0)
        for t in range(1, T):
            psy_t = psy_mm(t)
            ln_chain(t - 1, psy_prev)
            psy_prev = psy_t
        ln_chain(T - 1, psy_prev)


def kernel(**inputs):
    query = np.asarray(inputs["query"], np.float32)
    key_in = np.asarray(inputs["key_in"], np.float32)
    value = np.asarray(inputs["value"], np.float32)
    B = query.shape[0]
    assert query.shape == (B, S, DIM)

    if "nc" not in _CACHE:
        _CACHE["nc"] = build()
    nc = _CACHE["nc"]

    base = {
        "Wq": np.asarray(inputs["Wq"], np.float32),
        "Wv": np.asarray(inputs["Wv"], np.float32),
        "Wo": np.asarray(inputs["Wo"], np.float32),
        "bq": np.asarray(inputs["bq"], np.float32).reshape(1, DIM),
        "bv": np.asarray(inputs["bv"], np.float32).reshape(1, DIM),
        "bo": np.asarray(inputs["bo"], np.float32).reshape(1, DIM),
        "decay": np.asarray(inputs["decay"], np.float32).reshape(1, H),
        "gamma": np.asarray(inputs["gamma"], np.float32).reshape(1, DIM),
        "beta": np.asarray(inputs["beta"], np.float32).reshape(1, DIM),
    }
    in_maps = []
    for c in range(8):
        b = min(c, B - 1)
        m = dict(base)
        m["query"] = np.ascontiguousarray(query[b])
        m["key_in"] = np.ascontiguousarray(key_in[b])
        m["value"] = np.ascontiguousarray(value[b])
        in_maps.append(m)

    res = bass_utils.run_bass_kernel_spmd(nc, in_maps, core_ids=list(range(8)))
    out = np.stack([res.results[c]["out"] for c in range(B)], 0)
    return out.astype(np.float32)
